# revision 1
# baseline (speedup 1.0000x reference)
"""GTU (gated Toeplitz unit) Bass kernel for 8 TRN2 NeuronCores.

Sharding: tensor-parallel over heads (H=8 -> 1 head/core). Each core
computes its head's u/v projections, the RPE-MLP Toeplitz coefficients,
the causal depthwise long-conv via dense real-DFT matmuls (circular conv
of length 2n realized as TensorE matmuls with constant DFT matrices),
the gate, and a partial o-projection. Host sums the 8 partials + o_b.
"""

import numpy as np

B, N, E = 4, 2048, 1024
H = 8
D1 = 3 * E
DH = D1 // H            # 384
R = 512
GAMMA = 0.99
EPS = 1e-8
M2 = 2 * N              # 4096 (circular conv length)
KH = M2 // 2 + 1        # 2049 rfft bins
KP = 2176               # bins padded to 17*128
KA = 1024 + 128         # augmented contraction for x (bias row), 9*128
ROWS = B * N            # 8192

_CACHE = {}


def _t3(a):
    """(M, N) -> (128, M/128, N) partition-tiled layout."""
    m, n = a.shape
    assert m % 128 == 0
    return np.ascontiguousarray(
        a.reshape(m // 128, 128, n).transpose(1, 0, 2)).astype(np.float32)


def _from3(a):
    p, m, n = a.shape
    return np.ascontiguousarray(a.transpose(1, 0, 2)).reshape(m * 128, n)


def _consts():
    if "dft" in _CACHE:
        return _CACHE["dft"]
    l = np.arange(N, dtype=np.float64)[:, None]
    k = np.arange(KP, dtype=np.float64)[None, :]
    mask = (k < KH).astype(np.float64)
    ang = 2.0 * np.pi * l * k / M2
    cr = np.cos(ang) * mask
    ci = -np.sin(ang) * mask
    dft_cri = np.concatenate([cr, ci], axis=1)            # (2048, 4352)
    w = np.where((k[0] == 0) | (k[0] == M2 // 2), 1.0, 2.0) * mask[0]
    kk = np.arange(KP, dtype=np.float64)[:, None]
    t = np.arange(N, dtype=np.float64)[None, :]
    ang2 = 2.0 * np.pi * kk * t / M2
    icos = (w[:, None] / M2) * np.cos(ang2)               # (2176, 2048)
    isin = (-w[:, None] / M2) * np.sin(ang2)
    idft_cs = np.concatenate([icos, isin], axis=0)        # (4352, 2048)
    decay = GAMMA ** np.arange(N, dtype=np.float64)       # lag 0 -> 1.0
    decay_t = decay.reshape(N // 128, 128).T              # (128, 16)
    _CACHE["dft"] = (_t3(dft_cri), _t3(idft_cs), decay_t.astype(np.float32))
    return _CACHE["dft"]


def _build():
    import concourse.bass as bass
    import concourse.mybir as mybir
    import concourse.tile as tile
    from concourse import bacc
    from concourse.kernels.tile_matmul import matmul_tile_kernel

    AFT = mybir.ActivationFunctionType
    ALU = mybir.AluOpType
    dt = mybir.dt.float32

    nc = bacc.Bacc(None, target_bir_lowering=False, debug=False, num_devices=8)

    def din(name, shape):
        return nc.dram_tensor(name, list(shape), dt, kind="ExternalInput")

    def dint(name, shape):
        return nc.dram_tensor(name, list(shape), dt)

    xTa = din("xTa", (128, KA // 128, ROWS))
    u_wa = din("u_wa", (128, KA // 128, DH))
    v_wa = din("v_wa", (128, KA // 128, DH))
    o_w = din("o_w", (128, DH // 128, E))
    p_aug = din("p_aug", (2, N))
    pw_aug = din("pw_aug", (2, R))
    lws = [din(f"lw{i}", (128, R // 128, R)) for i in range(3)]
    lbs = din("lbs", (128, 3 * (R // 128)))   # 3 layers x (128, 4)
    out_w = din("out_w", (128, R // 128, DH))
    outb = din("outb", (1, DH))
    decay = din("decay", (128, N // 128))
    dft = din("dft", (128, N // 128, 2 * KP))
    idft = din("idft", (128, 2 * KP // 128, N))
    out = nc.dram_tensor("out", [128, ROWS // 128, E], dt, kind="ExternalOutput")

    acoef = dint("acoef", (128, N // 128, DH))
    arai = dint("arai", (128, 2 * KP // 128, DH))
    xrxi = dint("xrxi", (128, B * 2 * KP // 128, DH))
    prpi = dint("prpi", (128, B * 2 * KP // 128, DH))
    uT = dint("uT", (128, DH // 128, ROWS))
    v = dint("v", (128, ROWS // 128, DH))
    tvT = dint("tvT", (128, DH // 128, ROWS))
    gT = dint("gT", (128, DH // 128, ROWS))

    KG = KP // 128            # 17 freq groups
    FG = R // 128             # 4 feature groups

    def silu_evict(nc_, psum, sbuf):
        nc_.scalar.activation(sbuf, psum, AFT.Silu)

    with tile.TileContext(nc) as tc:
        # ---------------- RPE MLP (feature-major, fully in SBUF) --------
        with (tc.tile_pool(name="mlp", bufs=1) as mp,
              tc.tile_pool(name="mlp_ps", bufs=2, space="PSUM") as mps):
            ones_col = mp.tile([128, 1], dt)      # K=128 -> M=1 reducer
            nc.vector.memset(ones_col[:], 1.0)
            one_row = mp.tile([1, 128], dt)       # K=1 -> 128-partition bcast
            nc.vector.memset(one_row[:], 1.0)
            c_sc = mp.tile([1, 1], dt)
            nc.vector.memset(c_sc[:], float(R ** -0.5))
            eps_sc = mp.tile([1, 1], dt)
            nc.vector.memset(eps_sc[:], EPS)

            pa_sb = mp.tile([2, N], dt)
            pw_sb = mp.tile([2, R], dt)
            lb_sb = mp.tile([128, 3 * FG], dt)
            nc.sync.dma_start(pa_sb[:], p_aug[:])
            nc.sync.dma_start(pw_sb[:], pw_aug[:])
            nc.sync.dma_start(lb_sb[:], lbs[:])

            h = [mp.tile([128, N], dt, name=f"h{g}", tag=f"h{g}") for g in range(FG)]
            # h0 = pos_idx @ pos_w + pos_b   (K=2), feature-major (512, 2048)
            for g in range(FG):
                for nch in range(N // 512):
                    ps = mps.tile([128, 512], dt, name="mmps", tag="mm")
                    nc.tensor.matmul(
                        ps[:], pw_sb[:, g * 128:(g + 1) * 128],
                        pa_sb[:, nch * 512:(nch + 1) * 512],
                        start=True, stop=True)
                    nc.vector.tensor_copy(h[g][:, nch * 512:(nch + 1) * 512], ps[:])

            def srms_relu(h_in, phi_out):
                # s[t] = sum_f h^2 ; factor = 1/(sqrt(s)/sqrt(R) + eps)
                sq = [mp.tile([128, N], dt, name=f"sq{g}", tag=f"sq{g}") for g in range(FG)]
                for g in range(FG):
                    nc.vector.tensor_mul(sq[g][:], h_in[g][:], h_in[g][:])
                fac = mp.tile([1, N], dt, name="fac", tag="fac")
                for nch in range(N // 512):
                    ps1 = mps.tile([1, 512], dt, name="redps", tag="red")
                    for g in range(FG):
                        nc.tensor.matmul(
                            ps1[:], ones_col[:],
                            sq[g][:, nch * 512:(nch + 1) * 512],
                            start=(g == 0), stop=(g == FG - 1))
                    sl = fac[:, nch * 512:(nch + 1) * 512]
                    nc.scalar.activation(sl, ps1[:], AFT.Sqrt)
                    nc.vector.tensor_scalar(
                        sl, sl, c_sc[:], eps_sc[:], ALU.mult, ALU.add)
                    nc.vector.reciprocal(sl, sl)
                fb = mp.tile([128, N], dt, name="fb", tag="fb")
                for nch in range(N // 512):
                    psb = mps.tile([128, 512], dt, name="bcps", tag="bc")
                    nc.tensor.matmul(
                        psb[:], one_row[:], fac[:, nch * 512:(nch + 1) * 512],
                        start=True, stop=True)
                    nc.vector.tensor_copy(fb[:, nch * 512:(nch + 1) * 512], psb[:])
                for g in range(FG):
                    nc.vector.tensor_mul(phi_out[g][:], h_in[g][:], fb[:])
                    nc.scalar.activation(phi_out[g][:], phi_out[g][:], AFT.Relu)

            phi = [mp.tile([128, N], dt, name=f"phi{g}", tag=f"phi{g}") for g in range(FG)]
            srms_relu(h, phi)

            lw_sb = mp.tile([128, FG, R], dt)
            for li in range(3):
                nc.sync.dma_start(lw_sb[:], lws[li][:])
                for g in range(FG):
                    for nch in range(N // 512):
                        ps = mps.tile([128, 512], dt, name="mmps", tag="mm")
                        for k in range(FG):
                            nc.tensor.matmul(
                                ps[:], lw_sb[:, k, g * 128:(g + 1) * 128],
                                phi[k][:, nch * 512:(nch + 1) * 512],
                                start=(k == 0), stop=(k == FG - 1))
                        sl = h[g][:, nch * 512:(nch + 1) * 512]
                        nc.vector.tensor_scalar(
                            sl, ps[:], lb_sb[:, li * FG + g:li * FG + g + 1],
                            None, ALU.add)
                srms_relu(h, phi)

            # coefs (t-major) = phi.T @ out_w  -> * decay + out_b -> acoef
            ow_sb = mp.tile([128, FG, DH], dt)
            ob_sb = mp.tile([1, DH], dt)
            dec_sb = mp.tile([128, N // 128], dt)
            nc.sync.dma_start(ow_sb[:], out_w[:])
            nc.sync.dma_start(ob_sb[:], outb[:])
            nc.sync.dma_start(dec_sb[:], decay[:])
            obb = mp.tile([128, DH], dt)
            psb = mps.tile([128, DH], dt, name="bc2ps", tag="bc")
            nc.tensor.matmul(psb[:], one_row[:], ob_sb[:], start=True, stop=True)
            nc.vector.tensor_copy(obb[:], psb[:])
            for m in range(N // 128):
                ps = mps.tile([128, DH], dt, name="mm2ps", tag="mm")
                for k in range(FG):
                    nc.tensor.matmul(
                        ps[:], phi[k][:, m * 128:(m + 1) * 128],
                        ow_sb[:, k, :], start=(k == 0), stop=(k == FG - 1))
                ac = mp.tile([128, DH], dt, name="ac", tag="ac")
                nc.vector.tensor_add(ac[:], ps[:], obb[:])
                nc.vector.tensor_scalar(
                    ac[:], ac[:], dec_sb[:, m:m + 1], None, ALU.mult)
                nc.sync.dma_start(acoef[:, m, :], ac[:])

        # ---------------- big matmuls via matmul_tile_kernel ------------
        # A: kernel spectrum  ArAi = dft.T @ acoef   (K=2048, M=4352, N=384)
        matmul_tile_kernel(tc, dft[:], acoef[:], arai[:])
        # B: uT = silu(u_wa.T @ xTa)                 (K=1152, M=384, N=8192)
        matmul_tile_kernel(tc, u_wa[:], xTa[:], uT[:], psum_evict_fn=silu_evict)
        # C: v = silu(xTa.T @ v_wa)                  (K=1152, M=8192, N=384)
        matmul_tile_kernel(tc, xTa[:], v_wa[:], v[:], psum_evict_fn=silu_evict)
        # D: forward DFT of v per batch
        for b in range(B):
            matmul_tile_kernel(
                tc, dft[:],
                v[:, b * (N // 128):(b + 1) * (N // 128), :],
                xrxi[:, b * 2 * KG:(b + 1) * 2 * KG, :])

        # E: pointwise complex multiply  P = A * X
        with (tc.tile_pool(name="pw", bufs=1) as pwp,
              tc.tile_pool(name="pw2", bufs=4) as pw2):
            ar_sb = pwp.tile([128, 2 * KG, DH], dt)
            nc.sync.dma_start(ar_sb[:], arai[:])
            for b in range(B):
                for g in range(KG):
                    xr = pw2.tile([128, DH], dt, name="xr", tag="xr")
                    xi = pw2.tile([128, DH], dt, name="xi", tag="xi")
                    nc.sync.dma_start(xr[:], xrxi[:, b * 2 * KG + g, :])
                    nc.sync.dma_start(xi[:], xrxi[:, b * 2 * KG + KG + g, :])
                    pr = pw2.tile([128, DH], dt, name="pr", tag="pr")
                    pi = pw2.tile([128, DH], dt, name="pi", tag="pi")
                    t1 = pw2.tile([128, DH], dt, name="t1", tag="t1")
                    nc.vector.tensor_mul(pr[:], ar_sb[:, g, :], xr[:])
                    nc.vector.tensor_mul(t1[:], ar_sb[:, KG + g, :], xi[:])
                    nc.vector.tensor_sub(pr[:], pr[:], t1[:])
                    nc.vector.tensor_mul(pi[:], ar_sb[:, g, :], xi[:])
                    nc.vector.tensor_mul(t1[:], ar_sb[:, KG + g, :], xr[:])
                    nc.vector.tensor_add(pi[:], pi[:], t1[:])
                    nc.sync.dma_start(prpi[:, b * 2 * KG + g, :], pr[:])
                    nc.sync.dma_start(prpi[:, b * 2 * KG + KG + g, :], pi[:])

        # F: inverse DFT  tvT_b = PrPi_b.T @ idft_cs  (K=4352, M=384, N=2048)
        for b in range(B):
            matmul_tile_kernel(
                tc, prpi[:, b * 2 * KG:(b + 1) * 2 * KG, :], idft[:],
                tvT[:, :, b * N:(b + 1) * N])

        # G: gate  gT = uT * tvT
        with tc.tile_pool(name="gate", bufs=4) as gp:
            for m in range(DH // 128):
                for nch in range(ROWS // 2048):
                    ut = gp.tile([128, 2048], dt, name="ut", tag="ut")
                    tt = gp.tile([128, 2048], dt, name="tt", tag="tt")
                    nc.sync.dma_start(ut[:], uT[:, m, nch * 2048:(nch + 1) * 2048])
                    nc.sync.dma_start(tt[:], tvT[:, m, nch * 2048:(nch + 1) * 2048])
                    nc.vector.tensor_mul(ut[:], ut[:], tt[:])
                    nc.sync.dma_start(gT[:, m, nch * 2048:(nch + 1) * 2048], ut[:])

        # H: partial o-projection  out = gT.T @ o_w  (K=384, M=8192, N=1024)
        matmul_tile_kernel(tc, gT[:], o_w[:], out[:])

    nc.compile()
    return nc


def _get_nc():
    if "nc" not in _CACHE:
        _CACHE["nc"] = _build()
    return _CACHE["nc"]


def kernel(x, u_w, u_b, v_w, v_b, o_w, o_b,
           pos_w, pos_b, lw0, lb0, lw1, lb1, lw2, lb2, out_w, out_b):
    from concourse.bass_utils import run_bass_kernel_spmd

    dft3, idft3, decay_t = _consts()
    x_flat = np.asarray(x, np.float32).reshape(ROWS, E)
    xTa = np.zeros((KA, ROWS), np.float32)
    xTa[:E] = x_flat.T
    xTa[E] = 1.0
    xTa3 = _t3(xTa)

    p_aug = np.stack([np.arange(N, dtype=np.float32),
                      np.ones(N, np.float32)])
    pw_aug = np.concatenate([pos_w, pos_b[None, :]], 0).astype(np.float32)
    # lbs layout: [:, li*4 + g] = lb_li[g*128 + p]
    lbs = np.concatenate(
        [lb.reshape(R // 128, 128).T for lb in (lb0, lb1, lb2)],
        axis=1).astype(np.float32)

    in_maps = []
    for h in range(H):
        sl = slice(h * DH, (h + 1) * DH)
        u_wa = np.zeros((KA, DH), np.float32)
        u_wa[:E] = u_w[:, sl]
        u_wa[E] = u_b[sl]
        v_wa = np.zeros((KA, DH), np.float32)
        v_wa[:E] = v_w[:, sl]
        v_wa[E] = v_b[sl]
        in_maps.append(dict(
            xTa=xTa3, u_wa=_t3(u_wa), v_wa=_t3(v_wa),
            o_w=_t3(np.ascontiguousarray(o_w[sl, :]).astype(np.float32)),
            p_aug=p_aug, pw_aug=pw_aug,
            lw0=_t3(lw0.astype(np.float32)), lw1=_t3(lw1.astype(np.float32)),
            lw2=_t3(lw2.astype(np.float32)), lbs=lbs,
            out_w=_t3(np.ascontiguousarray(out_w[:, sl]).astype(np.float32)),
            outb=np.ascontiguousarray(out_b[None, sl]).astype(np.float32),
            decay=decay_t, dft=dft3, idft=idft3,
        ))

    nc = _get_nc()
    res = run_bass_kernel_spmd(nc, in_maps, core_ids=list(range(8)),
                               trace=bool(_CACHE.get("trace")))
    _CACHE["last_res"] = res
    acc = np.zeros((ROWS, E), np.float32)
    for i in range(H):
        acc += _from3(res.results[i]["out"])
    acc += o_b[None, :]
    return acc.reshape(B, N, E)



# revision 5
# speedup vs baseline: 2.2999x; 2.2999x over previous
"""GTU (gated Toeplitz unit) Bass kernel for 8 TRN2 NeuronCores.

Sharding: tensor-parallel over heads (H=8 -> 1 head/core). Each core
computes its head's u/v projections, the RPE-MLP Toeplitz coefficients
(truncated to 512 lags; gamma^512 ~ 5.8e-3 rel), and the causal
depthwise long-conv via overlap-save: four 1024-point packed-real DFTs
realized as bf16 TensorE matmuls, pointwise complex multiply in SBUF,
inverse DFT, gate, and a partial o-projection. Host sums 8 partials.
"""

import numpy as np
import ml_dtypes

B, N, E = 4, 2048, 1024
H = 8
D1 = 3 * E
DH = D1 // H            # 384
R = 512
GAMMA = 0.99
EPS = 1e-8
TR = 512                # kernel truncation / chunk length
M2 = 1024               # circular conv length per window
NB = M2 // 2            # 512 packed rows per (Re, Im) block
KA = 1024 + 128         # augmented contraction for x (bias row), 9*128
ROWS = B * N            # 8192
NW = N // TR            # 4 windows

BF = ml_dtypes.bfloat16

_CACHE = {}


def _t3(a, dtype=BF):
    """(M, N) -> (128, M/128, N) partition-tiled layout."""
    m, n = a.shape
    assert m % 128 == 0
    return np.ascontiguousarray(
        a.reshape(m // 128, 128, n).transpose(1, 0, 2)).astype(dtype)


def _from3(a):
    p, m, n = a.shape
    return np.ascontiguousarray(
        a.astype(np.float32).transpose(1, 0, 2)).reshape(m * 128, n)


def _consts():
    if "dft" in _CACHE:
        return _CACHE["dft"]
    t = np.arange(M2, dtype=np.float64)[:, None]
    k = np.arange(NB, dtype=np.float64)[None, :]
    ang = 2.0 * np.pi * t * k / M2
    dre = np.cos(ang)
    dim = -np.sin(ang)
    dim[:, 0] = np.cos(np.pi * t[:, 0])           # Nyquist in Im slot 0
    dfw = np.concatenate([dre, dim], axis=1)      # (1024, 1024)
    tt = np.arange(TR, dtype=np.float64)[None, :] + NB
    kk = np.arange(NB, dtype=np.float64)[:, None]
    ang2 = 2.0 * np.pi * kk * tt / M2
    ire = (2.0 / M2) * np.cos(ang2)
    ire[0] = 1.0 / M2
    iim = (-2.0 / M2) * np.sin(ang2)
    iim[0] = (1.0 / M2) * np.cos(np.pi * tt[0])
    imw = np.concatenate([ire, iim], axis=0)      # (1024, 512)
    decay = GAMMA ** np.arange(TR, dtype=np.float64)
    decay_t = decay.reshape(TR // 128, 128).T     # (128, 4)
    _CACHE["dft"] = (_t3(dfw), _t3(imw), decay_t.astype(np.float32))
    return _CACHE["dft"]


def _build(debug=False, sim_silu=False):
    import concourse.bass as bass  # noqa: F401
    import concourse.mybir as mybir
    import concourse.tile as tile
    from concourse import bacc

    AFT = mybir.ActivationFunctionType
    ALU = mybir.AluOpType
    f32 = mybir.dt.float32
    bf16 = mybir.dt.bfloat16

    nc = bacc.Bacc(None, target_bir_lowering=False, debug=debug, num_devices=8)

    def din(name, shape, dt=bf16):
        return nc.dram_tensor(name, list(shape), dt, kind="ExternalInput")

    xTa = din("xTa", (128, KA // 128, ROWS))
    u_wa = din("u_wa", (128, KA // 128, DH))
    v_wa = din("v_wa", (128, KA // 128, DH))
    o_w = din("o_w", (128, DH // 128, E))
    dfw = din("dfw", (128, M2 // 128, M2))
    imw = din("imw", (128, M2 // 128, TR))
    p_aug = din("p_aug", (2, TR), f32)
    pw_aug = din("pw_aug", (2, R), f32)
    lws = din("lws", (128, 3 * (R // 128), R), f32)
    lbs = din("lbs", (128, 3 * (R // 128)), f32)
    out_w = din("out_w", (128, R // 128, DH), f32)
    outb = din("outb", (1, DH), f32)
    decay = din("decay", (128, TR // 128), f32)
    out = nc.dram_tensor("out", [128, ROWS // 128, E], bf16,
                         kind="ExternalOutput")
    uT_d = nc.dram_tensor("uT_d", [128, DH // 128, ROWS], bf16)

    FG = R // 128             # 4 feature groups (MLP)
    KT = M2 // 128            # 8 packed-row tiles
    LT = TR // 128            # 4 lag tiles

    with tile.TileContext(nc) as tc:
        with (tc.tile_pool(name="persist", bufs=1) as pp,
              tc.tile_pool(name="ps512", bufs=4, space="PSUM") as psp,
              tc.tile_pool(name="ps1024", bufs=2, space="PSUM") as pso):
            # resident constants
            dfw_sb = pp.tile([128, KT, M2], bf16)
            imw_sb = pp.tile([128, KT, TR], bf16)
            uw_sb = pp.tile([128, KA // 128, DH], bf16)
            vw_sb = pp.tile([128, KA // 128, DH], bf16)
            ow_sb = pp.tile([128, DH // 128, E], bf16)
            nc.sync.dma_start(dfw_sb[:], dfw[:])
            nc.sync.dma_start(imw_sb[:], imw[:])
            nc.sync.dma_start(uw_sb[:], u_wa[:])
            nc.sync.dma_start(vw_sb[:], v_wa[:])
            nc.sync.dma_start(ow_sb[:], o_w[:])

            acoef = pp.tile([128, LT, DH], bf16)     # decayed coefs, lags 0..511
            A_sb = pp.tile([128, KT, DH], bf16)      # kernel spectrum (packed)
            a_ny = pp.tile([1, DH], bf16)
            vbuf = pp.tile([128, N // 128, B * DH], bf16)   # v, t-tiled, (b,d) cols

            # ---------------- RPE MLP (feature-major, positions 0..TR-1) ----
            with tc.tile_pool(name="mlp", bufs=1) as mp, \
                 tc.tile_pool(name="mlp2", bufs=2) as mp2:
                ones_col = mp.tile([128, 1], f32)
                nc.vector.memset(ones_col[:], 1.0)
                one_row = mp.tile([1, 128], f32)
                nc.vector.memset(one_row[:], 1.0)
                c_sc = mp.tile([1, 1], f32)
                nc.vector.memset(c_sc[:], float(R ** -0.5))
                eps_sc = mp.tile([1, 1], f32)
                nc.vector.memset(eps_sc[:], EPS)

                pa_sb = mp.tile([2, TR], f32)
                pw_sb = mp.tile([2, R], f32)
                lb_sb = mp.tile([128, 3 * FG], f32)
                nc.sync.dma_start(pa_sb[:], p_aug[:])
                nc.sync.dma_start(pw_sb[:], pw_aug[:])
                nc.sync.dma_start(lb_sb[:], lbs[:])

                h = [mp.tile([128, TR], f32, name=f"h{g}", tag=f"h{g}")
                     for g in range(FG)]
                # h0 = pos_idx @ pos_w + pos_b   (K=2), feature-major
                for g in range(FG):
                    ps = psp.tile([128, TR], f32, name="mmps", tag="ps")
                    nc.tensor.matmul(
                        ps[:], pw_sb[:, g * 128:(g + 1) * 128], pa_sb[:],
                        start=True, stop=True)
                    nc.vector.tensor_copy(h[g][:], ps[:])

                def srms_relu(h_in, phi_out):
                    sq = [mp.tile([128, TR], f32, name=f"sq{g}", tag=f"sq{g}")
                          for g in range(FG)]
                    for g in range(FG):
                        nc.vector.tensor_mul(sq[g][:], h_in[g][:], h_in[g][:])
                    fac = mp.tile([1, TR], f32, name="fac", tag="fac")
                    ps1 = psp.tile([1, TR], f32, name="redps", tag="ps")
                    for g in range(FG):
                        nc.tensor.matmul(
                            ps1[:], ones_col[:], sq[g][:],
                            start=(g == 0), stop=(g == FG - 1))
                    nc.scalar.activation(fac[:], ps1[:], AFT.Sqrt)
                    nc.vector.tensor_scalar(
                        fac[:], fac[:], c_sc[:], eps_sc[:], ALU.mult, ALU.add)
                    nc.vector.reciprocal(fac[:], fac[:])
                    fb = mp.tile([128, TR], f32, name="fb", tag="fb")
                    psb = psp.tile([128, TR], f32, name="bcps", tag="ps")
                    nc.tensor.matmul(psb[:], one_row[:], fac[:],
                                     start=True, stop=True)
                    nc.vector.tensor_copy(fb[:], psb[:])
                    for g in range(FG):
                        nc.vector.tensor_mul(phi_out[g][:], h_in[g][:], fb[:])
                        nc.scalar.activation(phi_out[g][:], phi_out[g][:],
                                             AFT.Relu)

                phi = [mp.tile([128, TR], f32, name=f"phi{g}", tag=f"phi{g}")
                       for g in range(FG)]
                srms_relu(h, phi)

                for li in range(3):
                    lw_sb = mp2.tile([128, FG, R], f32, tag="lw")
                    nc.sync.dma_start(lw_sb[:], lws[:, li * FG:(li + 1) * FG, :])
                    for g in range(FG):
                        ps = psp.tile([128, TR], f32, name="mmps", tag="ps")
                        for k in range(FG):
                            nc.tensor.matmul(
                                ps[:], lw_sb[:, k, g * 128:(g + 1) * 128],
                                phi[k][:], start=(k == 0), stop=(k == FG - 1))
                        nc.vector.tensor_scalar(
                            h[g][:], ps[:], lb_sb[:, li * FG + g:li * FG + g + 1],
                            None, ALU.add)
                    srms_relu(h, phi)

                # coefs (t-major) = phi.T @ out_w -> +out_b, *decay -> acoef
                ow2_sb = mp.tile([128, FG, DH], f32)
                ob_sb = mp.tile([1, DH], f32)
                dec_sb = mp.tile([128, LT], f32)
                nc.sync.dma_start(ow2_sb[:], out_w[:])
                nc.sync.dma_start(ob_sb[:], outb[:])
                nc.sync.dma_start(dec_sb[:], decay[:])
                obb = mp.tile([128, DH], f32)
                psb2 = psp.tile([128, DH], f32, name="bc2ps", tag="ps")
                nc.tensor.matmul(psb2[:], one_row[:], ob_sb[:],
                                 start=True, stop=True)
                nc.vector.tensor_copy(obb[:], psb2[:])
                for m in range(LT):
                    ps = psp.tile([128, DH], f32, name="mm2ps", tag="ps")
                    for k in range(FG):
                        nc.tensor.matmul(
                            ps[:], phi[k][:, m * 128:(m + 1) * 128],
                            ow2_sb[:, k, :], start=(k == 0), stop=(k == FG - 1))
                    ac = mp2.tile([128, DH], f32, name="ac", tag="ac")
                    nc.vector.tensor_add(ac[:], ps[:], obb[:])
                    nc.vector.tensor_scalar(
                        acoef[:, m, :], ac[:], dec_sb[:, m:m + 1], None,
                        ALU.mult)

            # ---------------- kernel spectrum A = D_f[0:512].T @ acoef ------
            for mb in range(KT):
                ps = psp.tile([128, DH], f32, name="ksps", tag="ps")
                for k in range(LT):
                    nc.tensor.matmul(
                        ps[:], dfw_sb[:, k, mb * 128:(mb + 1) * 128],
                        acoef[:, k, :], start=(k == 0), stop=(k == LT - 1))
                nc.scalar.activation(A_sb[:, mb, :], ps[:], AFT.Copy)
            nc.vector.tensor_copy(a_ny[:], A_sb[0:1, KT // 2, :])
            nc.vector.memset(A_sb[0:1, KT // 2, :], 0.0)

            # ---------------- u/v projections (x streamed once) -------------
            with tc.tile_pool(name="xs", bufs=3) as xsp, \
                 tc.tile_pool(name="stage", bufs=4) as stp:
                NCH = ROWS // 512      # 16 chunks of 512 rows
                for c in range(NCH):
                    xc = xsp.tile([128, KA // 128, 512], bf16, tag="xc")
                    nc.sync.dma_start(xc[:], xTa[:, :, c * 512:(c + 1) * 512])
                    for mu in range(DH // 128):
                        ps = psp.tile([128, 512], f32, name="ups", tag="ps")
                        for k in range(KA // 128):
                            nc.tensor.matmul(
                                ps[:], uw_sb[:, k, mu * 128:(mu + 1) * 128],
                                xc[:, k, :], start=(k == 0),
                                stop=(k == KA // 128 - 1))
                        ut = stp.tile([128, 512], bf16, tag="ut_st")
                        if sim_silu:
                            nc.scalar.activation(ut[:], ps[:], AFT.Sigmoid)
                            nc.vector.tensor_mul(ut[:], ut[:], ps[:])
                        else:
                            nc.scalar.activation(ut[:], ps[:], AFT.Silu)
                        nc.sync.dma_start(
                            uT_d[:, mu, c * 512:(c + 1) * 512], ut[:])
                    b, t4 = c // 4, (c % 4) * 4
                    for mv in range(4):
                        ps = psp.tile([128, DH], f32, name="vps", tag="ps")
                        for k in range(KA // 128):
                            nc.tensor.matmul(
                                ps[:], xc[:, k, mv * 128:(mv + 1) * 128],
                                vw_sb[:, k, :], start=(k == 0),
                                stop=(k == KA // 128 - 1))
                        vsl = vbuf[:, t4 + mv, b * DH:(b + 1) * DH]
                        if sim_silu:
                            nc.scalar.activation(vsl, ps[:], AFT.Sigmoid)
                            nc.vector.tensor_mul(vsl, vsl, ps[:])
                        else:
                            nc.scalar.activation(vsl, ps[:], AFT.Silu)

            # ---------------- conv windows + gate + o-projection ------------
            with tc.tile_pool(name="xw", bufs=2) as xwp, \
                 tc.tile_pool(name="tt", bufs=1) as ttp, \
                 tc.tile_pool(name="uin", bufs=4) as uip, \
                 tc.tile_pool(name="gw", bufs=6) as gwp, \
                 tc.tile_pool(name="ost", bufs=3) as osp:
                WDH = B * DH           # 1536
                for win in range(NW):
                    Xt = xwp.tile([128, KT, WDH], bf16, tag="X")
                    k0, nk = (LT, LT) if win == 0 else (0, KT)
                    vt0 = 0 if win == 0 else LT * (win - 1)
                    for mb in range(KT):
                        pss = [psp.tile([128, 512], f32, name=f"fps{c3}",
                                        tag="ps") for c3 in range(3)]
                        for k in range(nk):
                            for c3 in range(3):
                                nc.tensor.matmul(
                                    pss[c3][:],
                                    dfw_sb[:, k0 + k, mb * 128:(mb + 1) * 128],
                                    vbuf[:, vt0 + k, c3 * 512:(c3 + 1) * 512],
                                    start=(k == 0), stop=(k == nk - 1))
                        for c3 in range(3):
                            nc.scalar.activation(
                                Xt[:, mb, c3 * 512:(c3 + 1) * 512],
                                pss[c3][:], AFT.Copy)
                    # pointwise complex multiply (in place), per batch
                    HB = KT // 2
                    for b in range(B):
                        cs = slice(b * DH, (b + 1) * DH)
                        Xr = Xt[:, 0:HB, cs]
                        Xi = Xt[:, HB:KT, cs]
                        t1 = ttp.tile([128, HB, DH], bf16, tag="t1")
                        t2 = ttp.tile([128, HB, DH], bf16, tag="t2")
                        xny = ttp.tile([1, DH], bf16, tag="xny")
                        nc.vector.tensor_mul(t1[:], A_sb[:, HB:KT, :], Xi)
                        nc.vector.tensor_mul(t2[:], A_sb[:, HB:KT, :], Xr)
                        nc.vector.tensor_copy(xny[:], Xt[0:1, HB, cs])
                        nc.vector.tensor_mul(Xr, A_sb[:, 0:HB, :], Xr)
                        nc.vector.tensor_sub(Xr, Xr, t1[:])
                        nc.vector.tensor_mul(Xi, A_sb[:, 0:HB, :], Xi)
                        nc.vector.tensor_add(Xi, Xi, t2[:])
                        nc.vector.tensor_mul(Xt[0:1, HB, cs], a_ny[:], xny[:])
                    # inverse DFT + gate (per batch), then o-projection rows
                    for b in range(B):
                        gt = gwp.tile([128, DH // 128, 512], bf16, tag="g")
                        for md in range(DH // 128):
                            ut = uip.tile([128, 512], bf16, tag="uin")
                            nc.sync.dma_start(
                                ut[:],
                                uT_d[:, md,
                                     b * N + win * TR:b * N + win * TR + 512])
                            ps = psp.tile([128, 512], f32, name="ips", tag="ps")
                            for k in range(KT):
                                nc.tensor.matmul(
                                    ps[:],
                                    Xt[:, k, b * DH + md * 128:
                                       b * DH + (md + 1) * 128],
                                    imw_sb[:, k, :], start=(k == 0),
                                    stop=(k == KT - 1))
                            nc.vector.tensor_mul(gt[:, md, :], ps[:], ut[:])
                        # o-projection for these 512 rows (4 row-tiles)
                        r0 = (b * N + win * TR) // 128
                        for mr in range(4):
                            po = pso.tile([128, E], f32, tag="po")
                            for kd in range(DH // 128):
                                for n2 in range(E // 512):
                                    nc.tensor.matmul(
                                        po[:, n2 * 512:(n2 + 1) * 512],
                                        gt[:, kd, mr * 128:(mr + 1) * 128],
                                        ow_sb[:, kd, n2 * 512:(n2 + 1) * 512],
                                        start=(kd == 0),
                                        stop=(kd == DH // 128 - 1))
                            ost = osp.tile([128, E], bf16, tag="o_st")
                            nc.scalar.activation(ost[:], po[:], AFT.Copy)
                            nc.sync.dma_start(out[:, r0 + mr, :], ost[:])

    nc.compile()
    return nc


def _get_nc():
    if "nc" not in _CACHE:
        _CACHE["nc"] = _build()
    return _CACHE["nc"]


def _host_prep(x, u_w, u_b, v_w, v_b, o_w, o_b,
               pos_w, pos_b, lw0, lb0, lw1, lb1, lw2, lb2, out_w, out_b):
    dfw3, imw3, decay_t = _consts()
    x_flat = np.asarray(x, np.float32).reshape(ROWS, E)
    xTa = np.zeros((KA, ROWS), BF)
    xTa[:E] = x_flat.T.astype(BF)
    xTa[E] = 1.0
    xTa3 = _t3(xTa)

    p_aug = np.stack([np.arange(TR, dtype=np.float32),
                      np.ones(TR, np.float32)])
    pw_aug = np.concatenate([pos_w, pos_b[None, :]], 0).astype(np.float32)
    lws = np.concatenate(
        [_t3(lw.astype(np.float32), np.float32) for lw in (lw0, lw1, lw2)],
        axis=1)
    lbs = np.concatenate(
        [lb.reshape(R // 128, 128).T for lb in (lb0, lb1, lb2)],
        axis=1).astype(np.float32)

    in_maps = []
    for h in range(H):
        sl = slice(h * DH, (h + 1) * DH)
        u_wa = np.zeros((KA, DH), np.float32)
        u_wa[:E] = u_w[:, sl]
        u_wa[E] = u_b[sl]
        v_wa = np.zeros((KA, DH), np.float32)
        v_wa[:E] = v_w[:, sl]
        v_wa[E] = v_b[sl]
        in_maps.append(dict(
            xTa=xTa3, u_wa=_t3(u_wa), v_wa=_t3(v_wa),
            o_w=_t3(np.ascontiguousarray(o_w[sl, :]).astype(np.float32)),
            dfw=dfw3, imw=imw3,
            p_aug=p_aug, pw_aug=pw_aug, lws=lws, lbs=lbs,
            out_w=_t3(np.ascontiguousarray(out_w[:, sl]).astype(np.float32),
                      np.float32),
            outb=np.ascontiguousarray(out_b[None, sl]).astype(np.float32),
            decay=decay_t,
        ))
    return in_maps


def kernel(x, u_w, u_b, v_w, v_b, o_w, o_b,
           pos_w, pos_b, lw0, lb0, lw1, lb1, lw2, lb2, out_w, out_b):
    from concourse.bass_utils import run_bass_kernel_spmd

    in_maps = _host_prep(x, u_w, u_b, v_w, v_b, o_w, o_b,
                         pos_w, pos_b, lw0, lb0, lw1, lb1, lw2, lb2,
                         out_w, out_b)
    nc = _get_nc()
    res = run_bass_kernel_spmd(nc, in_maps, core_ids=list(range(8)),
                               trace=bool(_CACHE.get("trace")))
    _CACHE["last_res"] = res
    acc = np.zeros((ROWS, E), np.float32)
    for i in range(H):
        acc += _from3(res.results[i]["out"])
    acc += o_b[None, :]
    return acc.reshape(B, N, E)


# revision 11
# speedup vs baseline: 6.7590x; 2.9388x over previous
"""GTU (gated Toeplitz unit) Bass kernel for 8 TRN2 NeuronCores.

Sharding: tensor-parallel over heads (H=8 -> 1 head/core). Each core
computes its head's u/v projections, the RPE-MLP Toeplitz coefficients
(truncated to 512 lags; gamma^512 ~ 5.8e-3 rel), and the causal
depthwise long-conv via overlap-save with shared chunk spectra:
window spectrum X_i = S_{i-1} + (-1)^k S_i where S_j is the 1024-point
packed-real DFT of chunk j (Nyquist packed in the Im DC slot). All big
matmuls run in bf16 on TensorE with fp32 PSUM accumulation. Host sums
the 8 partial o-projections.
"""

import numpy as np
import ml_dtypes

B, N, E = 4, 2048, 1024
H = 8
D1 = 3 * E
DH = D1 // H            # 384
R = 512
GAMMA = 0.99
EPS = 1e-8
TR = 512                # kernel truncation / chunk length
M2 = 1024               # circular conv length per window
NB = M2 // 2            # 512 packed rows per (Re, Im) block
KA = 1024 + 128         # augmented contraction for x (bias row), 9*128
ROWS = B * N            # 8192
NW = N // TR            # 4 windows / chunks

BF = ml_dtypes.bfloat16

_CACHE = {}


def _t3(a, dtype=BF):
    """(M, N) -> (128, M/128, N) partition-tiled layout."""
    m, n = a.shape
    assert m % 128 == 0
    return np.ascontiguousarray(
        a.reshape(m // 128, 128, n).transpose(1, 0, 2)).astype(dtype)


def _from3(a):
    p, m, n = a.shape
    return np.ascontiguousarray(
        a.astype(np.float32).transpose(1, 0, 2)).reshape(m * 128, n)


def _consts():
    if "dft" in _CACHE:
        return _CACHE["dft"]
    t = np.arange(TR, dtype=np.float64)[:, None]       # only rows 0..511
    k = np.arange(NB, dtype=np.float64)[None, :]
    ang = 2.0 * np.pi * t * k / M2
    dre = np.cos(ang)
    dim = -np.sin(ang)
    dim[:, 0] = np.cos(np.pi * t[:, 0])           # Nyquist in Im slot 0
    dfw = np.concatenate([dre, dim], axis=1)      # (512, 1024)
    tt = np.arange(TR, dtype=np.float64)[None, :] + NB
    kk = np.arange(NB, dtype=np.float64)[:, None]
    ang2 = 2.0 * np.pi * kk * tt / M2
    ire = (2.0 / M2) * np.cos(ang2)
    ire[0] = 1.0 / M2
    iim = (-2.0 / M2) * np.sin(ang2)
    iim[0] = (1.0 / M2) * np.cos(np.pi * tt[0])
    imw = np.concatenate([ire, iim], axis=0)      # (1024, 512)
    decay = GAMMA ** np.arange(TR, dtype=np.float64)
    decay_t = decay.reshape(TR // 128, 128).T     # (128, 4)
    sgn = np.where(np.arange(128) % 2 == 0, 1.0, -1.0)[:, None]
    _CACHE["dft"] = (_t3(dfw), _t3(imw), decay_t.astype(np.float32),
                     sgn.astype(np.float32))
    return _CACHE["dft"]


def _build(debug=False, sim_silu=False):
    import concourse.bass as bass  # noqa: F401
    import concourse.mybir as mybir
    import concourse.tile as tile
    from concourse import bacc

    AFT = mybir.ActivationFunctionType
    ALU = mybir.AluOpType
    f32 = mybir.dt.float32
    bf16 = mybir.dt.bfloat16

    nc = bacc.Bacc(None, target_bir_lowering=False, debug=debug, num_devices=8)

    def din(name, shape, dt=bf16):
        return nc.dram_tensor(name, list(shape), dt, kind="ExternalInput")

    xTa = din("xTa", (128, KA // 128, ROWS))
    u_wa = din("u_wa", (128, KA // 128, DH))
    v_wa = din("v_wa", (128, KA // 128, DH))
    o_w = din("o_w", (128, DH // 128, E))
    dfw = din("dfw", (128, TR // 128, M2))
    imw = din("imw", (128, M2 // 128, TR))
    p_aug = din("p_aug", (2, TR), f32)
    pw_aug = din("pw_aug", (2, R), f32)
    lws = din("lws", (128, 3 * (R // 128), R))
    lbs = din("lbs", (128, 3 * (R // 128)), f32)
    out_w = din("out_w", (128, R // 128, DH))
    outb = din("outb", (1, DH), f32)
    decay = din("decay", (128, TR // 128), f32)
    sgn_in = din("sgn", (128, 1), f32)
    out = nc.dram_tensor("out", [128, ROWS // 128, E], bf16,
                         kind="ExternalOutput")
    uT_d = nc.dram_tensor("uT_d", [128, DH // 128, ROWS], bf16)

    FG = R // 128             # 4 feature groups (MLP)
    KT = M2 // 128            # 8 packed-row tiles
    LT = TR // 128            # 4 lag / chunk-time tiles
    HB = KT // 2              # 4 (Re block tiles)
    WDH = B * DH              # 1536

    with tile.TileContext(nc) as tc:
        with (tc.tile_pool(name="persist", bufs=1) as pp,
              tc.tile_pool(name="ps512", bufs=8, space="PSUM") as psp):
            # resident constants
            dfw_sb = pp.tile([128, LT, M2], bf16)
            imw_sb = pp.tile([128, KT, TR], bf16)
            uw_sb = pp.tile([128, KA // 128, DH], bf16)
            vw_sb = pp.tile([128, KA // 128, DH], bf16)
            ow_sb = pp.tile([128, DH // 128, E], bf16)
            sgn_sb = pp.tile([128, 1], f32)
            nc.sync.dma_start(dfw_sb[:], dfw[:])
            nc.sync.dma_start(imw_sb[:], imw[:])
            nc.sync.dma_start(uw_sb[:], u_wa[:])
            nc.sync.dma_start(vw_sb[:], v_wa[:])
            nc.sync.dma_start(ow_sb[:], o_w[:])
            nc.sync.dma_start(sgn_sb[:], sgn_in[:])

            acoef = pp.tile([128, LT, DH], bf16)     # decayed coefs, lags 0..511
            A_sb = pp.tile([128, KT, DH], bf16)      # kernel spectrum (packed)
            a_ny = pp.tile([1, DH], bf16)

            # ---------------- RPE MLP (feature-major, positions 0..TR-1) ----
            with tc.tile_pool(name="mlp", bufs=1) as mp, \
                 tc.tile_pool(name="mlp2", bufs=2) as mp2:
                ones_col = mp.tile([128, 1], f32)
                nc.vector.memset(ones_col[:], 1.0)
                one_row = mp.tile([1, 128], f32)
                nc.vector.memset(one_row[:], 1.0)
                c_sc = mp.tile([1, 1], f32)
                nc.vector.memset(c_sc[:], float(R ** -0.5))
                eps_sc = mp.tile([1, 1], f32)
                nc.vector.memset(eps_sc[:], EPS)

                pa_sb = mp.tile([2, TR], f32)
                pw_sb = mp.tile([2, R], f32)
                lb_sb = mp.tile([128, 3 * FG], f32)
                nc.sync.dma_start(pa_sb[:], p_aug[:])
                nc.sync.dma_start(pw_sb[:], pw_aug[:])
                nc.sync.dma_start(lb_sb[:], lbs[:])

                h = [mp.tile([128, TR], f32, name=f"h{g}", tag=f"h{g}")
                     for g in range(FG)]
                # h0 = pos_idx @ pos_w + pos_b   (K=2), feature-major, fp32
                for g in range(FG):
                    ps = psp.tile([128, TR], f32, name="mmps", tag="ps")
                    nc.tensor.matmul(
                        ps[:], pw_sb[:, g * 128:(g + 1) * 128], pa_sb[:],
                        start=True, stop=True)
                    nc.vector.tensor_copy(h[g][:], ps[:])

                def srms_relu(h_in, phi_out):
                    sq = [mp.tile([128, TR], f32, name=f"sq{g}", tag=f"sq{g}")
                          for g in range(FG)]
                    for g in range(FG):
                        nc.vector.tensor_mul(sq[g][:], h_in[g][:], h_in[g][:])
                    fac = mp.tile([1, TR], f32, name="fac", tag="fac")
                    ps1 = psp.tile([1, TR], f32, name="redps", tag="ps")
                    for g in range(FG):
                        nc.tensor.matmul(
                            ps1[:], ones_col[:], sq[g][:],
                            start=(g == 0), stop=(g == FG - 1))
                    nc.scalar.activation(fac[:], ps1[:], AFT.Sqrt)
                    nc.vector.tensor_scalar(
                        fac[:], fac[:], c_sc[:], eps_sc[:], ALU.mult, ALU.add)
                    nc.vector.reciprocal(fac[:], fac[:])
                    fb = mp.tile([128, TR], f32, name="fb", tag="fb")
                    psb = psp.tile([128, TR], f32, name="bcps", tag="ps")
                    nc.tensor.matmul(psb[:], one_row[:], fac[:],
                                     start=True, stop=True)
                    nc.vector.tensor_copy(fb[:], psb[:])
                    for g in range(FG):
                        nc.vector.tensor_mul(phi_out[g][:], h_in[g][:], fb[:])
                        nc.scalar.activation(phi_out[g][:], phi_out[g][:],
                                             AFT.Relu)

                # phi in bf16 so layer matmuls run at bf16 rate
                phi = [mp.tile([128, TR], bf16, name=f"phi{g}", tag=f"phi{g}")
                       for g in range(FG)]
                srms_relu(h, phi)

                for li in range(3):
                    lw_sb = mp2.tile([128, FG, R], bf16, tag="lw")
                    nc.sync.dma_start(lw_sb[:], lws[:, li * FG:(li + 1) * FG, :])
                    for g in range(FG):
                        ps = psp.tile([128, TR], f32, name="mmps", tag="ps")
                        for k in range(FG):
                            nc.tensor.matmul(
                                ps[:], lw_sb[:, k, g * 128:(g + 1) * 128],
                                phi[k][:], start=(k == 0), stop=(k == FG - 1))
                        nc.vector.tensor_scalar(
                            h[g][:], ps[:], lb_sb[:, li * FG + g:li * FG + g + 1],
                            None, ALU.add)
                    srms_relu(h, phi)

                # coefs (t-major) = phi.T @ out_w -> +out_b, *decay -> acoef
                ow2_sb = mp.tile([128, FG, DH], bf16)
                ob_sb = mp.tile([1, DH], f32)
                dec_sb = mp.tile([128, LT], f32)
                nc.sync.dma_start(ow2_sb[:], out_w[:])
                nc.sync.dma_start(ob_sb[:], outb[:])
                nc.sync.dma_start(dec_sb[:], decay[:])
                obb = mp.tile([128, DH], f32)
                psb2 = psp.tile([128, DH], f32, name="bc2ps", tag="ps")
                nc.tensor.matmul(psb2[:], one_row[:], ob_sb[:],
                                 start=True, stop=True)
                nc.vector.tensor_copy(obb[:], psb2[:])
                for m in range(LT):
                    ps = psp.tile([128, DH], f32, name="mm2ps", tag="ps")
                    for k in range(FG):
                        nc.tensor.matmul(
                            ps[:], phi[k][:, m * 128:(m + 1) * 128],
                            ow2_sb[:, k, :], start=(k == 0), stop=(k == FG - 1))
                    ac = mp2.tile([128, DH], f32, name="ac", tag="ac")
                    nc.vector.tensor_add(ac[:], ps[:], obb[:])
                    nc.vector.tensor_scalar(
                        acoef[:, m, :], ac[:], dec_sb[:, m:m + 1], None,
                        ALU.mult)

            # ---------------- kernel spectrum A = dfw.T @ acoef -------------
            for mb in range(KT):
                ps = psp.tile([128, DH], f32, name="ksps", tag="ps")
                for k in range(LT):
                    nc.tensor.matmul(
                        ps[:], dfw_sb[:, k, mb * 128:(mb + 1) * 128],
                        acoef[:, k, :], start=(k == 0), stop=(k == LT - 1))
                nc.scalar.activation(A_sb[:, mb, :], ps[:], AFT.Copy)
            nc.vector.tensor_copy(a_ny[:], A_sb[0:1, HB, :])
            nc.vector.memset(A_sb[0:1, HB, :], 0.0)

            # ------- fused u/v + chunk-DFT + pointwise + window pipeline ----
            with tc.tile_pool(name="xs", bufs=2) as xsp, \
                 tc.tile_pool(name="stage", bufs=4) as stp, \
                 tc.tile_pool(name="vb", bufs=2) as vbp, \
                 tc.tile_pool(name="spool", bufs=2) as spp, \
                 tc.tile_pool(name="ppool", bufs=2) as ppp, \
                 tc.tile_pool(name="tt", bufs=1) as ttp, \
                 tc.tile_pool(name="uin", bufs=4) as uip, \
                 tc.tile_pool(name="gw", bufs=4) as gwp, \
                 tc.tile_pool(name="ost", bufs=3) as osp:
                def emit_uv(j):
                    # u/v projections for time-group j (all batches)
                    vb = vbp.tile([128, LT, WDH], bf16, tag="vb")
                    for b4 in range(B):
                        c = j + 4 * b4
                        xc = xsp.tile([128, KA // 128, 512], bf16, tag="xc")
                        nc.sync.dma_start(xc[:],
                                          xTa[:, :, c * 512:(c + 1) * 512])
                        for mu in range(DH // 128):
                            ps = psp.tile([128, 512], f32, name="ups", tag="ps")
                            for k in range(KA // 128):
                                nc.tensor.matmul(
                                    ps[:], uw_sb[:, k, mu * 128:(mu + 1) * 128],
                                    xc[:, k, :], start=(k == 0),
                                    stop=(k == KA // 128 - 1))
                            ut = stp.tile([128, 512], bf16, tag="ut_st")
                            if sim_silu:
                                nc.scalar.activation(ut[:], ps[:], AFT.Sigmoid)
                                nc.vector.tensor_mul(ut[:], ut[:], ps[:])
                            else:
                                nc.scalar.activation(ut[:], ps[:], AFT.Silu)
                            nc.sync.dma_start(
                                uT_d[:, mu, c * 512:(c + 1) * 512], ut[:])
                        for mv in range(4):
                            ps = psp.tile([128, DH], f32, name="vps", tag="ps")
                            for k in range(KA // 128):
                                nc.tensor.matmul(
                                    ps[:], xc[:, k, mv * 128:(mv + 1) * 128],
                                    vw_sb[:, k, :], start=(k == 0),
                                    stop=(k == KA // 128 - 1))
                            vsl = vb[:, mv, b4 * DH:(b4 + 1) * DH]
                            if sim_silu:
                                nc.scalar.activation(vsl, ps[:], AFT.Sigmoid)
                                nc.vector.tensor_mul(vsl, vsl, ps[:])
                            else:
                                nc.scalar.activation(vsl, ps[:], AFT.Silu)
                    return vb

                S_prev = None
                vb = emit_uv(0)
                for j in range(NW):
                    # ---- chunk DFT: S_j = dfw.T @ v_chunk_j   (K=512)
                    S = spp.tile([128, KT, WDH], bf16, tag="S")
                    for mb in range(KT):
                        pss = [psp.tile([128, 512], f32, name=f"fps{c3}",
                                        tag="ps") for c3 in range(3)]
                        for k in range(LT):
                            for c3 in range(3):
                                nc.tensor.matmul(
                                    pss[c3][:],
                                    dfw_sb[:, k, mb * 128:(mb + 1) * 128],
                                    vb[:, k, c3 * 512:(c3 + 1) * 512],
                                    start=(k == 0), stop=(k == LT - 1))
                        for c3 in range(3):
                            nc.scalar.activation(
                                S[:, mb, c3 * 512:(c3 + 1) * 512],
                                pss[c3][:], AFT.Copy)
                    # ---- Q_j = A * S_j  (packed complex multiply, in place)
                    for b in range(B):
                        cs = slice(b * DH, (b + 1) * DH)
                        Sr = S[:, 0:HB, cs]
                        Si = S[:, HB:KT, cs]
                        t1 = ttp.tile([128, HB, DH], bf16, tag="t1")
                        t2 = ttp.tile([128, HB, DH], bf16, tag="t2")
                        sny = ttp.tile([1, DH], bf16, tag="sny")
                        nc.vector.tensor_mul(t1[:], A_sb[:, HB:KT, :], Si)
                        nc.vector.tensor_mul(t2[:], A_sb[:, HB:KT, :], Sr)
                        nc.vector.tensor_copy(sny[:], S[0:1, HB, cs])
                        nc.vector.tensor_mul(Sr, A_sb[:, 0:HB, :], Sr)
                        nc.vector.tensor_sub(Sr, Sr, t1[:])
                        nc.vector.tensor_mul(Si, A_sb[:, 0:HB, :], Si)
                        nc.vector.tensor_add(Si, Si, t2[:])
                        nc.vector.tensor_mul(S[0:1, HB, cs], a_ny[:], sny[:])
                    # next group's u/v matmuls fill PE while DVE does Q_j
                    if j + 1 < NW:
                        vb = emit_uv(j + 1)
                    # ---- window j: P = Q_{j-1} + (-1)^k Q_j, inverse, gate, o
                    for b in range(B):
                        cs = slice(b * DH, (b + 1) * DH)
                        P = ppp.tile([128, KT, DH], bf16, tag="P")
                        nc.vector.tensor_scalar(
                            P[:], S[:, :, cs], sgn_sb[:, 0:1], None, ALU.mult)
                        if S_prev is not None:
                            nc.vector.tensor_add(P[:], P[:],
                                                 S_prev[:, :, cs])
                        gt = gwp.tile([128, DH // 128, 512], bf16, tag="g")
                        for md in range(DH // 128):
                            ut = uip.tile([128, 512], bf16, tag="uin")
                            nc.sync.dma_start(
                                ut[:],
                                uT_d[:, md,
                                     b * N + j * TR:b * N + j * TR + 512])
                            ps = psp.tile([128, 512], f32, name="ips", tag="ps")
                            for k in range(KT):
                                nc.tensor.matmul(
                                    ps[:], P[:, k, md * 128:(md + 1) * 128],
                                    imw_sb[:, k, :], start=(k == 0),
                                    stop=(k == KT - 1))
                            nc.vector.tensor_mul(gt[:, md, :], ps[:], ut[:])
                        # o-projection for these 512 rows (4 row-tiles)
                        r0 = (b * N + j * TR) // 128
                        for mr in range(4):
                            for n2 in range(E // 512):
                                po = psp.tile([128, 512], f32, name="ops",
                                              tag="ps")
                                for kd in range(DH // 128):
                                    nc.tensor.matmul(
                                        po[:],
                                        gt[:, kd, mr * 128:(mr + 1) * 128],
                                        ow_sb[:, kd, n2 * 512:(n2 + 1) * 512],
                                        start=(kd == 0),
                                        stop=(kd == DH // 128 - 1))
                                ost = osp.tile([128, 512], bf16, tag="o_st")
                                nc.scalar.activation(ost[:], po[:], AFT.Copy)
                                nc.sync.dma_start(
                                    out[:, r0 + mr, n2 * 512:(n2 + 1) * 512],
                                    ost[:])
                    S_prev = S

    nc.compile()
    return nc


def _get_nc():
    if "nc" not in _CACHE:
        _CACHE["nc"] = _build()
    return _CACHE["nc"]


def _host_prep(x, u_w, u_b, v_w, v_b, o_w, o_b,
               pos_w, pos_b, lw0, lb0, lw1, lb1, lw2, lb2, out_w, out_b):
    dfw3, imw3, decay_t, sgn = _consts()
    x_flat = np.asarray(x, np.float32).reshape(ROWS, E)
    xTa = np.zeros((KA, ROWS), BF)
    xTa[:E] = x_flat.T.astype(BF)
    xTa[E] = 1.0
    xTa3 = _t3(xTa)

    p_aug = np.stack([np.arange(TR, dtype=np.float32),
                      np.ones(TR, np.float32)])
    pw_aug = np.concatenate([pos_w, pos_b[None, :]], 0).astype(np.float32)
    lws = np.concatenate(
        [_t3(lw.astype(np.float32)) for lw in (lw0, lw1, lw2)], axis=1)
    lbs = np.concatenate(
        [lb.reshape(R // 128, 128).T for lb in (lb0, lb1, lb2)],
        axis=1).astype(np.float32)

    in_maps = []
    for h in range(H):
        sl = slice(h * DH, (h + 1) * DH)
        u_wa = np.zeros((KA, DH), np.float32)
        u_wa[:E] = u_w[:, sl]
        u_wa[E] = u_b[sl]
        v_wa = np.zeros((KA, DH), np.float32)
        v_wa[:E] = v_w[:, sl]
        v_wa[E] = v_b[sl]
        in_maps.append(dict(
            xTa=xTa3, u_wa=_t3(u_wa), v_wa=_t3(v_wa),
            o_w=_t3(np.ascontiguousarray(o_w[sl, :]).astype(np.float32)),
            dfw=dfw3, imw=imw3,
            p_aug=p_aug, pw_aug=pw_aug, lws=lws, lbs=lbs,
            out_w=_t3(np.ascontiguousarray(out_w[:, sl]).astype(np.float32)),
            outb=np.ascontiguousarray(out_b[None, sl]).astype(np.float32),
            decay=decay_t, sgn=sgn,
        ))
    return in_maps


def kernel(x, u_w, u_b, v_w, v_b, o_w, o_b,
           pos_w, pos_b, lw0, lb0, lw1, lb1, lw2, lb2, out_w, out_b):
    from concourse.bass_utils import run_bass_kernel_spmd

    in_maps = _host_prep(x, u_w, u_b, v_w, v_b, o_w, o_b,
                         pos_w, pos_b, lw0, lb0, lw1, lb1, lw2, lb2,
                         out_w, out_b)
    nc = _get_nc()
    res = run_bass_kernel_spmd(nc, in_maps, core_ids=list(range(8)),
                               trace=bool(_CACHE.get("trace")))
    _CACHE["last_res"] = res
    acc = np.zeros((ROWS, E), np.float32)
    for i in range(H):
        acc += _from3(res.results[i]["out"])
    acc += o_b[None, :]
    return acc.reshape(B, N, E)
